# revision 10
# baseline (speedup 1.0000x reference)
import os
import sys
import types

sys.path.insert(0, '/opt/trn_rl_repo')

import numpy as np
import ml_dtypes

BF16NP = ml_dtypes.bfloat16
F8NP = ml_dtypes.float8_e4m3

try:
    import antenv
    if 'antenv.axon_hooks' not in sys.modules:
        _m = types.ModuleType('antenv.axon_hooks')
        _hook_store = {}
        _m.set_axon_ntff_profile_hook = lambda h: _hook_store.__setitem__('h', h)
        _m.get_axon_ntff_profile_hook = lambda: _hook_store.get('h')
        sys.modules['antenv.axon_hooks'] = _m
        antenv.axon_hooks = _m
        try:
            from trn_agent_boot.trn_boot import _ntff_profile_via_ctypes
            _hook = _ntff_profile_via_ctypes('/opt/axon/libaxon_pjrt.so')
            if _hook is not None:
                _m.set_axon_ntff_profile_hook(_hook)
        except Exception:
            pass
except Exception:
    pass

import concourse.bass as bass
import concourse.mybir as mybir
from concourse import bacc
from concourse.tile import TileContext
from concourse import bass_utils

F32 = mybir.dt.float32
BF16 = mybir.dt.bfloat16
F8 = mybir.dt.float8e4
AF = mybir.ActivationFunctionType
ALU = mybir.AluOpType
DR = mybir.MatmulPerfMode.DoubleRow

P = 128
D = 2048
F = 8192
E = 8
R = 16
NCORES = 8
T_FULL = 4096
TC = T_FULL // NCORES
DKT = D // P
FT = F // P
DT_TILES = D // P
SCALE = 64.0
INV = 1.0 / SCALE

LAST_RESULT = {}
_NC_CACHE = {}


def build_nc():
    if 'nc' in _NC_CACHE:
        return _NC_CACHE['nc']
    nc = bacc.Bacc(None, target_bir_lowering=False)

    xt_d = nc.dram_tensor("xt", [D, TC], F32, kind="ExternalInput")
    wgx_d = nc.dram_tensor("wgx", [FT, P, DKT, 2, P], F8, kind="ExternalInput")
    wux_d = nc.dram_tensor("wux", [FT, P, DKT, 2, P], F8, kind="ExternalInput")
    wdx_d = nc.dram_tensor("wdx", [DT_TILES, 4, P, DKT, 2, P], F8, kind="ExternalInput")
    rwt_d = nc.dram_tensor("rwt", [D, E], F32, kind="ExternalInput")
    agp_d = nc.dram_tensor("agp", [D, E * R], BF16, kind="ExternalInput")
    aup_d = nc.dram_tensor("aup", [D, E * R], BF16, kind="ExternalInput")
    pmw_d = nc.dram_tensor("pmw", [64, E, F], BF16, kind="ExternalInput")
    adt_d = nc.dram_tensor("adt", [F, E, P], BF16, kind="ExternalInput")
    bd2_d = nc.dram_tensor("bd2", [E * R, D], BF16, kind="ExternalInput")
    oneh_d = nc.dram_tensor("oneh", [E, E, P], BF16, kind="ExternalInput")
    idt_d = nc.dram_tensor("idt", [P, P], F32, kind="ExternalInput")
    out_d = nc.dram_tensor("outT", [D, TC], F32, kind="ExternalOutput")

    with TileContext(nc) as tc:
        with tc.tile_pool(name="big", bufs=1) as big, \
             tc.tile_pool(name="wstream", bufs=4) as wstream, \
             tc.tile_pool(name="xstream", bufs=1) as xstream, \
             tc.tile_pool(name="adtp", bufs=1) as adtp, \
             tc.tile_pool(name="ebuf", bufs=2) as ebuf, \
             tc.tile_pool(name="whbuf", bufs=6) as whbuf, \
             tc.tile_pool(name="gpsbuf", bufs=3) as gpsbuf, \
             tc.tile_pool(name="hbuf", bufs=2) as hbuf, \
             tc.tile_pool(name="obuf", bufs=2) as obuf, \
             tc.tile_pool(name="ppg", bufs=3, space="PSUM") as ppg, \
             tc.tile_pool(name="ppu", bufs=2, space="PSUM") as ppu, \
             tc.tile_pool(name="ppt", bufs=1, space="PSUM") as ppt, \
             tc.tile_pool(name="ppk", bufs=1, space="PSUM") as ppk:

            xhl = big.tile([P, DKT, 2, TC], F8, name="xhl")
            xtr = big.tile([P, DKT, TC], BF16, name="xtr")
            hbar8 = big.tile([P, FT, 2, TC], F8, name="hbar8")
            wb = big.tile([P, E, TC], BF16, name="wb")
            spm = big.tile([P, E, TC], BF16, name="spm")
            pmt2 = [big.tile([P, 2, E, P], BF16, name=f"pmt{i}") for i in range(2)]
            psb = big.tile([P, TC], BF16, name="psb")
            bd2s = big.tile([P, D], BF16, name="bd2s")
            w8 = big.tile([E, 4, P], BF16, name="w8")
            rw = big.tile([P, DKT, E], F32, name="rw")
            oneh = big.tile([E, E, P], BF16, name="oneh")
            idt = big.tile([P, P], F32, name="idt")
            scr = big.tile([P, 16], F32, name="scr")
            wtl = big.tile([P, 4, E], F32, name="wtl")

            nc.sync.dma_start(rw, rwt_d.rearrange("(kt p) e -> p kt e", p=P))
            nc.sync.dma_start(idt, idt_d[:, :])
            for q in range(2, 4):
                nc.gpsimd.memset(spm[32 * q:32 * (q + 1), :, :], 0.0)
            for i in range(2):
                for q in range(1, 4):
                    nc.gpsimd.memset(pmt2[i][32 * q:32 * (q + 1), 0, :, :], 0.0)
                nc.gpsimd.memset(pmt2[i][0:32, 1, :, :], 0.0)
                for q in range(2, 4):
                    nc.gpsimd.memset(pmt2[i][32 * q:32 * (q + 1), 1, :, :], 0.0)

            pbank = [ppk.tile([P, TC], F32, name=f"pbank{i}") for i in range(2)]

            pslT = ppt.tile([P, TC], F32, name="trans")
            for tt in range(4):
                xt_tt = xstream.tile([P, DKT, P], F32, name="xchunk")
                nc.sync.dma_start(
                    xt_tt, xt_d[:, bass.ts(tt, P)].rearrange("(kt p) t -> p kt t", p=P))
                nc.vector.tensor_copy(xtr[:, :, bass.ts(tt, P)], xt_tt)
                nc.vector.tensor_copy(xhl[:, :, 1, bass.ts(tt, P)], xt_tt)
                nc.vector.scalar_tensor_tensor(
                    xhl[:, :, 0, bass.ts(tt, P)], xhl[:, :, 1, bass.ts(tt, P)],
                    -1.0, xt_tt, op0=ALU.mult, op1=ALU.add)
                for kt in range(DKT):
                    nc.tensor.matmul(pslT[0:E, bass.ts(tt, P)], rw[:, kt, :],
                                     xt_tt[:, kt, :], start=(kt == 0),
                                     stop=(kt == DKT - 1))
            lsT = big.tile([E, TC], F32, name="lsT")
            nc.vector.tensor_copy(lsT, pslT[0:E, :])
            for tt in range(4):
                psl = ppt.tile([P, TC], F32, name="trans")
                nc.tensor.transpose(psl[:, 0:E], lsT[:, bass.ts(tt, P)],
                                    idt[0:E, 0:E])
                nmx = scr[:, 1:2]
                mx = scr[:, 0:1]
                m2 = scr[:, 2:3]
                rcp = scr[:, 3:4]
                z = scr[:, 4:12]
                lcp = wtl[:, 0, :]
                nc.vector.tensor_reduce(nmx, psl[:, 0:E], axis=mybir.AxisListType.X,
                                        op=ALU.max, negate=True)
                nc.vector.tensor_scalar_mul(mx, nmx, -1.0)
                nc.scalar.activation(z, psl[:, 0:E], AF.Exp, bias=nmx)
                lm1 = wtl[:, 1, :]
                nc.vector.tensor_scalar(lm1, psl[:, 0:E], mx, -1e30,
                                        op0=ALU.is_ge, op1=ALU.mult)
                nc.vector.tensor_tensor(lcp, psl[:, 0:E], lm1, op=ALU.add)
                nc.vector.tensor_reduce(m2, lcp, axis=mybir.AxisListType.X, op=ALU.max)
                wsel = wtl[:, 1, :]
                nc.vector.scalar_tensor_tensor(wsel, psl[:, 0:E], m2, z,
                                               op0=ALU.is_ge, op1=ALU.mult)
                nc.vector.tensor_reduce(rcp, wsel, axis=mybir.AxisListType.X, op=ALU.add)
                nc.vector.reciprocal(rcp, rcp)
                wcur = wtl[:, 2 + (tt % 2), :]
                nc.vector.tensor_scalar_mul(wcur, wsel, rcp)
                psw = ppt.tile([P, TC], F32, name="trans")
                nc.tensor.transpose(psw[0:E, 0:P], wcur, idt)
                nc.vector.tensor_copy(w8[:, tt, :], psw[0:E, 0:P])
            nc.sync.dma_start(oneh, oneh_d[:, :, :])
            w8flat = w8.rearrange("p a b -> p (a b)")
            for e in range(E):
                pswb = ppt.tile([P, TC], F32, name="trans")
                nc.tensor.matmul(pswb, oneh[:, e, :], w8flat, start=True, stop=True)
                nc.vector.tensor_copy(wb[:, e, :], pswb)

            for gi, src in enumerate((agp_d, aup_d)):
                ap_t = xstream.tile([P, DKT, P], BF16, name="apchunk")
                nc.sync.dma_start(ap_t, src.rearrange("(kt p) m -> p kt m", p=P))
                sps = ppu.tile([P, TC], F32, name="banku")
                for kt in range(DKT):
                    nc.tensor.matmul(sps, ap_t[:, kt, :], xtr[:, kt, :],
                                     start=(kt == 0), stop=(kt == DKT - 1))
                s_stage = ebuf.tile([P, TC], BF16, name="t1")
                nc.vector.tensor_copy(s_stage, sps)
                base = 32 * gi
                nc.gpsimd.memset(spm[base:base + R, 0, :], 0.0)
                for e in range(E):
                    if e >= 1:
                        nc.sync.dma_start(spm[base:base + R, e, :],
                                          s_stage[(e - 1) * R:e * R, :])
                    nc.sync.dma_start(spm[base + R:base + 2 * R, e, :],
                                      s_stage[e * R:(e + 1) * R, :])

            for f in range(FT):
                wg_t = wstream.tile([P, DKT, 2, P], F8, name="wchunk")
                nc.sync.dma_start(wg_t, wgx_d[f])
                wu_t = wstream.tile([P, DKT, 2, P], F8, name="wchunk")
                nc.sync.dma_start(wu_t, wux_d[f])
                pm_t = pmt2[f % 2]
                nc.sync.dma_start(pm_t[0:32, 0, :, :], pmw_d[0:32, :, bass.ts(f, P)])
                nc.sync.dma_start(pm_t[32:64, 1, :, :], pmw_d[32:64, :, bass.ts(f, P)])
                if f % 4 == 0:
                    adt_t = adtp.tile([P, 4, E, P], BF16, name="adt")
                    nc.sync.dma_start(
                        adt_t, adt_d[f * P:(f + 4) * P, :, :].rearrange(
                            "(fo p) e r -> p fo e r", p=P))

                bank_g = ppg.tile([P, TC], F32, name="bankg")
                bank_u = ppu.tile([P, TC], F32, name="banku")
                for i in range(DKT // 2):
                    nc.tensor.matmul(bank_g, wg_t[:, 2 * i:2 * i + 2, 0, :],
                                     xhl[:, 2 * i:2 * i + 2, 1, :],
                                     start=(i == 0), stop=False, perf_mode=DR)
                for i in range(DKT):
                    nc.tensor.matmul(bank_g, wg_t[:, i, :, :], xhl[:, i, :, :],
                                     start=False, stop=False, perf_mode=DR)
                nc.tensor.matmul(bank_g, pm_t[:, 0, 0, :], spm[:, 0, :],
                                 start=False, stop=False)
                for i in range(DKT // 2):
                    nc.tensor.matmul(bank_u, wu_t[:, 2 * i:2 * i + 2, 0, :],
                                     xhl[:, 2 * i:2 * i + 2, 1, :],
                                     start=(i == 0), stop=False, perf_mode=DR)
                for i in range(DKT):
                    nc.tensor.matmul(bank_u, wu_t[:, i, :, :], xhl[:, i, :, :],
                                     start=False, stop=False, perf_mode=DR)
                nc.tensor.matmul(bank_u, pm_t[:, 1, 0, :], spm[:, 0, :],
                                 start=False, stop=False)

                ht = hbuf.tile([P, TC], BF16, name="ht")
                for e in range(E):
                    s_act = ebuf.tile([P, TC], BF16, name="sact")
                    nc.scalar.activation(s_act, bank_g, AF.Silu, scale=INV)
                    if e + 1 < E:
                        nc.tensor.matmul(bank_g, pm_t[:, 0, e + 1, :],
                                         spm[:, e + 1, :],
                                         start=False, stop=(e + 1 == E - 1))
                    t1 = whbuf.tile([P, TC], BF16, name="wh")
                    nc.vector.scalar_tensor_tensor(t1, bank_u, INV, s_act,
                                                   op0=ALU.mult, op1=ALU.mult)
                    if e + 1 < E:
                        nc.tensor.matmul(bank_u, pm_t[:, 1, e + 1, :],
                                         spm[:, e + 1, :],
                                         start=False, stop=(e + 1 == E - 1))
                    nc.tensor.matmul(pbank[e // 4],
                                     adt_t[:, f % 4, e, :], t1,
                                     start=(f == 0 and e % 4 == 0),
                                     stop=(f == FT - 1 and e % 4 == 3))
                    wv = gpsbuf.tile([P, TC], BF16, name="whw")
                    nc.vector.tensor_tensor(wv, t1, wb[:, e, :], op=ALU.mult)
                    if e == 1:
                        nc.gpsimd.tensor_tensor(ht, wv0, wv, op=ALU.add)
                    elif e > 1:
                        nc.gpsimd.tensor_tensor(ht, ht, wv, op=ALU.add)
                    wv0 = wv
                nc.scalar.activation(hbar8[:, f, 1, :], ht, AF.Copy)
                nc.gpsimd.tensor_tensor(hbar8[:, f, 0, :], ht, hbar8[:, f, 1, :],
                                        op=ALU.subtract)

            for b in range(2):
                p_stage = ebuf.tile([P, TC], BF16, name="t1")
                for eo in range(4):
                    e = b * 4 + eo
                    nc.vector.scalar_tensor_tensor(
                        p_stage[32 * eo:32 * eo + R, :],
                        pbank[b][32 * eo:32 * eo + R, :], 1.0,
                        wb[0:R, e, :], op0=ALU.bypass, op1=ALU.mult)
                    nc.sync.dma_start(psb[e * R:(e + 1) * R, :],
                                      p_stage[32 * eo:32 * eo + R, :])

            nc.sync.dma_start(bd2s, bd2_d[:, :])
            for d in range(DT_TILES):
                psd = ppg.tile([P, TC], F32, name="bankg")
                for fc in range(4):
                    wd_t = wstream.tile([P, DKT, 2, P], F8, name="wdchunk")
                    nc.sync.dma_start(wd_t, wdx_d[d, fc])
                    fb = fc * DKT
                    for i in range(DKT // 2):
                        nc.tensor.matmul(psd, wd_t[:, 2 * i:2 * i + 2, 0, :],
                                         hbar8[:, fb + 2 * i:fb + 2 * i + 2, 1, :],
                                         start=(fc == 0 and i == 0), stop=False,
                                         perf_mode=DR)
                    for i in range(DKT):
                        nc.tensor.matmul(psd, wd_t[:, i, :, :],
                                         hbar8[:, fb + i, :, :],
                                         start=False, stop=False, perf_mode=DR)
                nc.tensor.matmul(psd, bd2s[:, bass.ts(d, P)], psb,
                                 start=False, stop=True)
                o_t = obuf.tile([P, TC], F32, name="osb")
                nc.scalar.activation(o_t, psd, AF.Copy, scale=INV)
                nc.sync.dma_start(out_d[bass.ts(d, P), :], o_t)

    nc.finalize()
    _NC_CACHE['nc'] = nc
    return nc


def _split8(a):
    hi = a.astype(F8NP)
    lo = (a - hi.astype(np.float32)).astype(F8NP)
    return hi, lo


def _host_prep(hidden_states, router_w, Wg, Wu, Wd, Ag, Bg, Au, Bu, Ad, Bd):
    f32 = np.float32
    X = np.ascontiguousarray(hidden_states.reshape(T_FULL, D), dtype=f32)
    xT = np.ascontiguousarray(X.T)

    def packgu(Wt):
        hi, lo = _split8(np.ascontiguousarray(Wt, dtype=f32) * SCALE)
        out = np.empty((FT, P, DKT, 2, P), dtype=F8NP)
        out[:, :, :, 0, :] = hi.reshape(DKT, P, FT, P).transpose(2, 1, 0, 3)
        out[:, :, :, 1, :] = lo.reshape(DKT, P, FT, P).transpose(2, 1, 0, 3)
        return np.ascontiguousarray(out)

    def packd(Wt):
        hi, lo = _split8(np.ascontiguousarray(Wt, dtype=f32) * SCALE)
        out = np.empty((DT_TILES, 4, P, DKT, 2, P), dtype=F8NP)
        out[:, :, :, :, 0, :] = hi.reshape(4, DKT, P, DT_TILES, P).transpose(3, 0, 2, 1, 4)
        out[:, :, :, :, 1, :] = lo.reshape(4, DKT, P, DT_TILES, P).transpose(3, 0, 2, 1, 4)
        return np.ascontiguousarray(out)

    shared = {
        "wgx": packgu(Wg.T),
        "wux": packgu(Wu.T),
        "wdx": packd(Wd.T),
        "rwt": np.ascontiguousarray(router_w.T, dtype=f32),
        "agp": np.ascontiguousarray(Ag.transpose(2, 0, 1).reshape(D, E * R)).astype(BF16NP),
        "aup": np.ascontiguousarray(Au.transpose(2, 0, 1).reshape(D, E * R)).astype(BF16NP),
    }
    pmw = np.zeros((64, E, F), dtype=f32)
    BgT = np.transpose(Bg, (0, 2, 1))
    BuT = np.transpose(Bu, (0, 2, 1))
    for e in range(E):
        if e >= 1:
            pmw[0:R, e] = -2.0 * SCALE * BgT[e - 1]
            pmw[32:48, e] = -2.0 * SCALE * BuT[e - 1]
        pmw[R:32, e] = 2.0 * SCALE * BgT[e]
        pmw[48:64, e] = 2.0 * SCALE * BuT[e]
    shared["pmw"] = pmw.astype(BF16NP)
    adt = np.zeros((F, E, P), dtype=f32)
    AdT = Ad.transpose(2, 0, 1)
    for e in range(E):
        adt[:, e, 32 * (e % 4):32 * (e % 4) + R] = AdT[:, e, :]
    shared["adt"] = adt.astype(BF16NP)
    shared["bd2"] = np.ascontiguousarray(
        (2.0 * SCALE * Bd.transpose(0, 2, 1)).reshape(E * R, D)).astype(BF16NP)
    oneh = np.zeros((E, E, P), dtype=f32)
    for e in range(E):
        oneh[e, e, :] = 1.0
    shared["oneh"] = oneh.astype(BF16NP)
    shared["idt"] = np.eye(P, dtype=f32)
    in_maps = []
    for c in range(NCORES):
        m = dict(shared)
        m["xt"] = np.ascontiguousarray(xT[:, c * TC:(c + 1) * TC])
        in_maps.append(m)
    return in_maps


def kernel(hidden_states, router_w, Wg, Wu, Wd, Ag, Bg, Au, Bu, Ad, Bd):
    hidden_states = np.asarray(hidden_states)
    nc = build_nc()
    in_maps = _host_prep(np.asarray(hidden_states, dtype=np.float32),
                         np.asarray(router_w), np.asarray(Wg), np.asarray(Wu),
                         np.asarray(Wd), np.asarray(Ag), np.asarray(Bg),
                         np.asarray(Au), np.asarray(Bu), np.asarray(Ad),
                         np.asarray(Bd))
    trace = bool(os.environ.get("TRNK_TRACE"))
    res = bass_utils.run_bass_kernel_spmd(
        nc, in_maps, core_ids=list(range(NCORES)), trace=trace)
    LAST_RESULT['exec_time_ns'] = res.exec_time_ns
    LAST_RESULT['res'] = res
    out = np.empty((T_FULL, D), dtype=np.float32)
    for c in range(NCORES):
        out[c * TC:(c + 1) * TC, :] = res.results[c]["outT"].T
    return out.reshape(hidden_states.shape[0], hidden_states.shape[1], D)


# revision 30
# speedup vs baseline: 1.2799x; 1.2799x over previous
import os
import sys
import types

sys.path.insert(0, '/opt/trn_rl_repo')

import numpy as np
import ml_dtypes

BF16NP = ml_dtypes.bfloat16
F8NP = ml_dtypes.float8_e4m3

try:
    import antenv
    if 'antenv.axon_hooks' not in sys.modules:
        _m = types.ModuleType('antenv.axon_hooks')
        _hook_store = {}
        _m.set_axon_ntff_profile_hook = lambda h: _hook_store.__setitem__('h', h)
        _m.get_axon_ntff_profile_hook = lambda: _hook_store.get('h')
        sys.modules['antenv.axon_hooks'] = _m
        antenv.axon_hooks = _m
        try:
            from trn_agent_boot.trn_boot import _ntff_profile_via_ctypes
            _hook = _ntff_profile_via_ctypes('/opt/axon/libaxon_pjrt.so')
            if _hook is not None:
                _m.set_axon_ntff_profile_hook(_hook)
        except Exception:
            pass
except Exception:
    pass

import concourse.bass as bass
import concourse.mybir as mybir
from concourse import bacc
from concourse.tile import TileContext
from concourse import bass_utils

F32 = mybir.dt.float32
BF16 = mybir.dt.bfloat16
F8 = mybir.dt.float8e4
AF = mybir.ActivationFunctionType
ALU = mybir.AluOpType
DRMODE = mybir.MatmulPerfMode.DoubleRow
UPSCALE = 64.0
UPINV = 1.0 / UPSCALE

P = 128
D = 2048
F = 8192
E = 8
R = 16
NCORES = 8
T_FULL = 4096
TC = T_FULL // NCORES
DKT = D // P
FT = F // P
DT_TILES = D // P

LAST_RESULT = {}
_NC_CACHE = {}


def build_nc():
    if 'nc' in _NC_CACHE:
        return _NC_CACHE['nc']
    nc = bacc.Bacc(None, target_bir_lowering=False)

    xt_d = nc.dram_tensor("xt", [D, TC], F32, kind="ExternalInput")
    wgt_d = nc.dram_tensor("wgt", [D, F], BF16, kind="ExternalInput")
    wux_d = nc.dram_tensor("wux", [FT, P, DKT, P], F8, kind="ExternalInput")
    wdt_d = nc.dram_tensor("wdt", [F, D], BF16, kind="ExternalInput")
    rwt_d = nc.dram_tensor("rwt", [D, E], F32, kind="ExternalInput")
    agp_d = nc.dram_tensor("agp", [D, E * R], BF16, kind="ExternalInput")
    aup_d = nc.dram_tensor("aup", [D, E * R], BF16, kind="ExternalInput")
    pmw_d = nc.dram_tensor("pmw", [64, E, F], BF16, kind="ExternalInput")
    adt_d = nc.dram_tensor("adt", [F, E, P], BF16, kind="ExternalInput")
    bd2_d = nc.dram_tensor("bd2", [E * R, D], BF16, kind="ExternalInput")
    oneh_d = nc.dram_tensor("oneh", [E, E, P], BF16, kind="ExternalInput")
    idt_d = nc.dram_tensor("idt", [P, P], F32, kind="ExternalInput")
    out_d = nc.dram_tensor("outT", [D, TC], F32, kind="ExternalOutput")

    with TileContext(nc) as tc:
        with tc.tile_pool(name="big", bufs=1) as big, \
             tc.tile_pool(name="wstream", bufs=3) as wstream, \
             tc.tile_pool(name="xstream", bufs=1) as xstream, \
             tc.tile_pool(name="adtp", bufs=1) as adtp, \
             tc.tile_pool(name="ebuf", bufs=2) as ebuf, \
             tc.tile_pool(name="whbuf", bufs=3) as whbuf, \
             tc.tile_pool(name="gpsbuf", bufs=5) as gpsbuf, \
             tc.tile_pool(name="obuf", bufs=2) as obuf, \
             tc.tile_pool(name="ppg", bufs=3, space="PSUM") as ppg, \
             tc.tile_pool(name="ppu", bufs=2, space="PSUM") as ppu, \
             tc.tile_pool(name="ppt", bufs=1, space="PSUM") as ppt, \
             tc.tile_pool(name="ppk", bufs=1, space="PSUM") as ppk:

            xtr = big.tile([P, DKT, TC], BF16, name="xtr")
            xf8 = big.tile([P, DKT, TC], F8, name="xf8")
            hbar = big.tile([P, FT, TC], BF16, name="hbar")
            wb = big.tile([P, E, TC], BF16, name="wb")
            spm = big.tile([P, E, TC], BF16, name="spm")
            pmt2 = [big.tile([P, 2, E, P], BF16, name=f"pmt{i}") for i in range(2)]
            psb = big.tile([P, TC], BF16, name="psb")
            bd2s = big.tile([P, D], BF16, name="bd2s")
            w8 = big.tile([E, 4, P], BF16, name="w8")
            rw = big.tile([P, DKT, E], F32, name="rw")
            oneh = big.tile([E, E, P], BF16, name="oneh")
            idt = big.tile([P, P], F32, name="idt")
            scr = big.tile([P, 16], F32, name="scr")
            wtl = big.tile([P, 4, E], F32, name="wtl")

            nc.sync.dma_start(rw, rwt_d.rearrange("(kt p) e -> p kt e", p=P))
            nc.sync.dma_start(idt, idt_d[:, :])
            for q in range(2, 4):
                nc.gpsimd.memset(spm[32 * q:32 * (q + 1), :, :], 0.0)
            for i in range(2):
                for q in range(1, 4):
                    nc.gpsimd.memset(pmt2[i][32 * q:32 * (q + 1), 0, :, :], 0.0)
                nc.gpsimd.memset(pmt2[i][0:32, 1, :, :], 0.0)
                for q in range(2, 4):
                    nc.gpsimd.memset(pmt2[i][32 * q:32 * (q + 1), 1, :, :], 0.0)

            pbank = [ppk.tile([P, TC], F32, name=f"pbank{i}") for i in range(2)]

            pslT = ppt.tile([P, TC], F32, name="trans")
            for tt in range(4):
                xt_tt = xstream.tile([P, DKT, P], F32, name="xchunk")
                nc.sync.dma_start(
                    xt_tt, xt_d[:, bass.ts(tt, P)].rearrange("(kt p) t -> p kt t", p=P))
                nc.vector.tensor_copy(xtr[:, :, bass.ts(tt, P)], xt_tt)
                nc.vector.tensor_copy(xf8[:, :, bass.ts(tt, P)], xt_tt)
                for kt in range(DKT):
                    nc.tensor.matmul(pslT[0:E, bass.ts(tt, P)], rw[:, kt, :],
                                     xt_tt[:, kt, :], start=(kt == 0),
                                     stop=(kt == DKT - 1))
            lsT = big.tile([E, TC], F32, name="lsT")
            nc.vector.tensor_copy(lsT, pslT[0:E, :])
            for tt in range(4):
                psl = ppt.tile([P, TC], F32, name="trans")
                nc.tensor.transpose(psl[:, 0:E], lsT[:, bass.ts(tt, P)],
                                    idt[0:E, 0:E])
                nmx = scr[:, 1:2]
                mx = scr[:, 0:1]
                m2 = scr[:, 2:3]
                rcp = scr[:, 3:4]
                z = scr[:, 4:12]
                lcp = wtl[:, 0, :]
                nc.vector.tensor_reduce(nmx, psl[:, 0:E], axis=mybir.AxisListType.X,
                                        op=ALU.max, negate=True)
                nc.vector.tensor_scalar_mul(mx, nmx, -1.0)
                nc.scalar.activation(z, psl[:, 0:E], AF.Exp, bias=nmx)
                lm1 = wtl[:, 1, :]
                nc.vector.tensor_scalar(lm1, psl[:, 0:E], mx, -1e30,
                                        op0=ALU.is_ge, op1=ALU.mult)
                nc.vector.tensor_tensor(lcp, psl[:, 0:E], lm1, op=ALU.add)
                nc.vector.tensor_reduce(m2, lcp, axis=mybir.AxisListType.X, op=ALU.max)
                wsel = wtl[:, 1, :]
                nc.vector.scalar_tensor_tensor(wsel, psl[:, 0:E], m2, z,
                                               op0=ALU.is_ge, op1=ALU.mult)
                nc.vector.tensor_reduce(rcp, wsel, axis=mybir.AxisListType.X, op=ALU.add)
                nc.vector.reciprocal(rcp, rcp)
                wcur = wtl[:, 2 + (tt % 2), :]
                nc.vector.tensor_scalar_mul(wcur, wsel, rcp)
                psw = ppt.tile([P, TC], F32, name="trans")
                nc.tensor.transpose(psw[0:E, 0:P], wcur, idt)
                nc.vector.tensor_copy(w8[:, tt, :], psw[0:E, 0:P])
            nc.sync.dma_start(oneh, oneh_d[:, :, :])
            w8flat = w8.rearrange("p a b -> p (a b)")
            for e in range(E):
                pswb = ppt.tile([P, TC], F32, name="trans")
                nc.tensor.matmul(pswb, oneh[:, e, :], w8flat, start=True, stop=True)
                nc.vector.tensor_copy(wb[:, e, :], pswb)

            for gi, src in enumerate((agp_d, aup_d)):
                ap_t = xstream.tile([P, DKT, P], BF16, name="apchunk")
                nc.sync.dma_start(ap_t, src.rearrange("(kt p) m -> p kt m", p=P))
                sps = ppu.tile([P, TC], F32, name="banku")
                for kt in range(DKT):
                    nc.tensor.matmul(sps, ap_t[:, kt, :], xtr[:, kt, :],
                                     start=(kt == 0), stop=(kt == DKT - 1))
                s_stage = ebuf.tile([P, TC], BF16, name="t1")
                nc.vector.tensor_copy(s_stage, sps)
                base = 32 * gi
                nc.gpsimd.memset(spm[base:base + R, 0, :], 0.0)
                for e in range(E):
                    if e >= 1:
                        nc.sync.dma_start(spm[base:base + R, e, :],
                                          s_stage[(e - 1) * R:e * R, :])
                    nc.sync.dma_start(spm[base + R:base + 2 * R, e, :],
                                      s_stage[e * R:(e + 1) * R, :])

            for f in range(FT):
                wg_t = wstream.tile([P, DKT, P], BF16, name="wchunk")
                nc.sync.dma_start(wg_t, wgt_d[:, bass.ts(f, P)].rearrange("(kt p) m -> p kt m", p=P))
                wu_t = wstream.tile([P, DKT, P], F8, name="wchunk8")
                nc.sync.dma_start(wu_t, wux_d[f])
                pm_t = pmt2[f % 2]
                nc.sync.dma_start(pm_t[0:32, 0, :, :], pmw_d[0:32, :, bass.ts(f, P)])
                nc.sync.dma_start(pm_t[32:64, 1, :, :], pmw_d[32:64, :, bass.ts(f, P)])
                if f % 4 == 0:
                    adt_t = adtp.tile([P, 4, E, P], BF16, name="adt")
                    nc.sync.dma_start(
                        adt_t, adt_d[f * P:(f + 4) * P, :, :].rearrange(
                            "(fo p) e r -> p fo e r", p=P))

                bank_g = ppg.tile([P, TC], F32, name="bankg")
                bank_u = ppu.tile([P, TC], F32, name="banku")
                for kt in range(DKT):
                    nc.tensor.matmul(bank_g, wg_t[:, kt, :], xtr[:, kt, :],
                                     start=(kt == 0), stop=False)
                nc.tensor.matmul(bank_g, pm_t[:, 0, 0, :], spm[:, 0, :],
                                 start=False, stop=False)
                for i in range(DKT // 2):
                    nc.tensor.matmul(bank_u, wu_t[:, 2 * i:2 * i + 2, :],
                                     xf8[:, 2 * i:2 * i + 2, :],
                                     start=(i == 0), stop=False, perf_mode=DRMODE)
                nc.tensor.matmul(bank_u, pm_t[:, 1, 0, :], spm[:, 0, :],
                                 start=False, stop=False)

                wvp = []
                t1p = None
                for e in range(E):
                    s_act = ebuf.tile([P, TC], BF16, name="sact")
                    nc.scalar.activation(s_act, bank_g, AF.Silu)
                    if e + 1 < E:
                        nc.tensor.matmul(bank_g, pm_t[:, 0, e + 1, :],
                                         spm[:, e + 1, :],
                                         start=False, stop=(e + 1 == E - 1))
                    if e % 2 == 0:
                        t1p = whbuf.tile([P, 2, TC], BF16, name="wh")
                    t1 = t1p[:, e % 2, :]
                    nc.vector.scalar_tensor_tensor(t1, bank_u, UPINV, s_act,
                                                   op0=ALU.mult, op1=ALU.mult)
                    if e + 1 < E:
                        nc.tensor.matmul(bank_u, pm_t[:, 1, e + 1, :],
                                         spm[:, e + 1, :],
                                         start=False, stop=(e + 1 == E - 1))
                    nc.tensor.matmul(pbank[e // 4],
                                     adt_t[:, f % 4, e, :], t1,
                                     start=(f == 0 and e % 4 == 0),
                                     stop=(f == FT - 1 and e % 4 == 3))
                    if e % 2 == 1:
                        wv = gpsbuf.tile([P, 2, TC], BF16, name="whw")
                        nc.vector.tensor_tensor(wv, t1p, wb[:, e - 1:e + 1, :],
                                                op=ALU.mult)
                        wvp.append(wv)
                a1 = gpsbuf.tile([P, 2, TC], BF16, name="whw")
                nc.gpsimd.tensor_tensor(a1, wvp[0], wvp[1], op=ALU.add)
                a2 = gpsbuf.tile([P, 2, TC], BF16, name="whw")
                nc.gpsimd.tensor_tensor(a2, wvp[2], wvp[3], op=ALU.add)
                nc.gpsimd.tensor_tensor(a1, a1, a2, op=ALU.add)
                nc.gpsimd.tensor_tensor(hbar[:, f, :], a1[:, 0, :], a1[:, 1, :],
                                        op=ALU.add)

            for b in range(2):
                p_stage = ebuf.tile([P, TC], BF16, name="t1")
                for eo in range(4):
                    e = b * 4 + eo
                    nc.vector.scalar_tensor_tensor(
                        p_stage[32 * eo:32 * eo + R, :],
                        pbank[b][32 * eo:32 * eo + R, :], 1.0,
                        wb[0:R, e, :], op0=ALU.bypass, op1=ALU.mult)
                    nc.sync.dma_start(psb[e * R:(e + 1) * R, :],
                                      p_stage[32 * eo:32 * eo + R, :])

            nc.sync.dma_start(bd2s, bd2_d[:, :])
            for d in range(DT_TILES):
                psd = ppg.tile([P, TC], F32, name="bankg")
                for fc in range(4):
                    wd_t = wstream.tile([P, DKT, P], BF16, name="wdchunk")
                    nc.sync.dma_start(
                        wd_t, wdt_d[fc * 2048:(fc + 1) * 2048, bass.ts(d, P)].rearrange(
                            "(kt p) m -> p kt m", p=P))
                    for kt in range(DKT):
                        nc.tensor.matmul(psd, wd_t[:, kt, :], hbar[:, fc * DKT + kt, :],
                                         start=(fc == 0 and kt == 0), stop=False)
                nc.tensor.matmul(psd, bd2s[:, bass.ts(d, P)], psb,
                                 start=False, stop=True)
                o_t = obuf.tile([P, TC], F32, name="osb")
                nc.scalar.activation(o_t, psd, AF.Copy)
                nc.sync.dma_start(out_d[bass.ts(d, P), :], o_t)

    nc.finalize()
    _NC_CACHE['nc'] = nc
    return nc


def _host_prep(hidden_states, router_w, Wg, Wu, Wd, Ag, Bg, Au, Bu, Ad, Bd):
    f32 = np.float32
    X = np.ascontiguousarray(hidden_states.reshape(T_FULL, D), dtype=f32)
    xT = np.ascontiguousarray(X.T)
    wus = (np.ascontiguousarray(Wu.T, dtype=f32) * UPSCALE).astype(F8NP)
    wux = np.ascontiguousarray(
        wus.reshape(DKT, P, FT, P).transpose(2, 1, 0, 3))
    shared = {
        "wgt": np.ascontiguousarray(Wg.T).astype(BF16NP),
        "wux": wux,
        "wdt": np.ascontiguousarray(Wd.T).astype(BF16NP),
        "rwt": np.ascontiguousarray(router_w.T, dtype=f32),
        "agp": np.ascontiguousarray(Ag.transpose(2, 0, 1).reshape(D, E * R)).astype(BF16NP),
        "aup": np.ascontiguousarray(Au.transpose(2, 0, 1).reshape(D, E * R)).astype(BF16NP),
    }
    pmw = np.zeros((64, E, F), dtype=f32)
    BgT = np.transpose(Bg, (0, 2, 1))
    BuT = np.transpose(Bu, (0, 2, 1))
    for e in range(E):
        if e >= 1:
            pmw[0:R, e] = -2.0 * BgT[e - 1]
            pmw[32:48, e] = -2.0 * UPSCALE * BuT[e - 1]
        pmw[R:32, e] = 2.0 * BgT[e]
        pmw[48:64, e] = 2.0 * UPSCALE * BuT[e]
    shared["pmw"] = pmw.astype(BF16NP)
    adt = np.zeros((F, E, P), dtype=f32)
    AdT = Ad.transpose(2, 0, 1)
    for e in range(E):
        adt[:, e, 32 * (e % 4):32 * (e % 4) + R] = AdT[:, e, :]
    shared["adt"] = adt.astype(BF16NP)
    shared["bd2"] = np.ascontiguousarray(
        (2.0 * Bd.transpose(0, 2, 1)).reshape(E * R, D)).astype(BF16NP)
    oneh = np.zeros((E, E, P), dtype=f32)
    for e in range(E):
        oneh[e, e, :] = 1.0
    shared["oneh"] = oneh.astype(BF16NP)
    shared["idt"] = np.eye(P, dtype=f32)
    in_maps = []
    for c in range(NCORES):
        m = dict(shared)
        m["xt"] = np.ascontiguousarray(xT[:, c * TC:(c + 1) * TC])
        in_maps.append(m)
    return in_maps


def kernel(hidden_states, router_w, Wg, Wu, Wd, Ag, Bg, Au, Bu, Ad, Bd):
    hidden_states = np.asarray(hidden_states)
    nc = build_nc()
    in_maps = _host_prep(np.asarray(hidden_states, dtype=np.float32),
                         np.asarray(router_w), np.asarray(Wg), np.asarray(Wu),
                         np.asarray(Wd), np.asarray(Ag), np.asarray(Bg),
                         np.asarray(Au), np.asarray(Bu), np.asarray(Ad),
                         np.asarray(Bd))
    trace = bool(os.environ.get("TRNK_TRACE"))
    res = bass_utils.run_bass_kernel_spmd(
        nc, in_maps, core_ids=list(range(NCORES)), trace=trace)
    LAST_RESULT['exec_time_ns'] = res.exec_time_ns
    LAST_RESULT['res'] = res
    out = np.empty((T_FULL, D), dtype=np.float32)
    for c in range(NCORES):
        out[c * TC:(c + 1) * TC, :] = res.results[c]["outT"].T
    return out.reshape(hidden_states.shape[0], hidden_states.shape[1], D)



# revision 38
# speedup vs baseline: 1.2801x; 1.0001x over previous
import os
import sys
import types

sys.path.insert(0, '/opt/trn_rl_repo')

import numpy as np
import ml_dtypes

BF16NP = ml_dtypes.bfloat16
F8NP = ml_dtypes.float8_e4m3

try:
    import antenv
    if 'antenv.axon_hooks' not in sys.modules:
        _m = types.ModuleType('antenv.axon_hooks')
        _hook_store = {}
        _m.set_axon_ntff_profile_hook = lambda h: _hook_store.__setitem__('h', h)
        _m.get_axon_ntff_profile_hook = lambda: _hook_store.get('h')
        sys.modules['antenv.axon_hooks'] = _m
        antenv.axon_hooks = _m
        try:
            from trn_agent_boot.trn_boot import _ntff_profile_via_ctypes
            _hook = _ntff_profile_via_ctypes('/opt/axon/libaxon_pjrt.so')
            if _hook is not None:
                _m.set_axon_ntff_profile_hook(_hook)
        except Exception:
            pass
except Exception:
    pass

import concourse.bass as bass
import concourse.mybir as mybir
from concourse import bacc
from concourse.tile import TileContext
from concourse import bass_utils

F32 = mybir.dt.float32
BF16 = mybir.dt.bfloat16
F8 = mybir.dt.float8e4
AF = mybir.ActivationFunctionType
ALU = mybir.AluOpType
DRMODE = mybir.MatmulPerfMode.DoubleRow
UPSCALE = 64.0
UPINV = 1.0 / UPSCALE

P = 128
D = 2048
F = 8192
E = 8
R = 16
NCORES = 8
T_FULL = 4096
TC = T_FULL // NCORES
DKT = D // P
FT = F // P
DT_TILES = D // P

LAST_RESULT = {}
_NC_CACHE = {}


def build_nc():
    if 'nc' in _NC_CACHE:
        return _NC_CACHE['nc']
    nc = bacc.Bacc(None, target_bir_lowering=False)

    xt_d = nc.dram_tensor("xt", [D, TC], F32, kind="ExternalInput")
    wgt_d = nc.dram_tensor("wgt", [D, F], BF16, kind="ExternalInput")
    wux_d = nc.dram_tensor("wux", [FT, P, DKT, P], F8, kind="ExternalInput")
    wdt_d = nc.dram_tensor("wdt", [F, D], BF16, kind="ExternalInput")
    rwt_d = nc.dram_tensor("rwt", [D, E], F32, kind="ExternalInput")
    agp_d = nc.dram_tensor("agp", [D, E * R], BF16, kind="ExternalInput")
    aup_d = nc.dram_tensor("aup", [D, E * R], BF16, kind="ExternalInput")
    pmw_d = nc.dram_tensor("pmw", [64, E, F], BF16, kind="ExternalInput")
    adt_d = nc.dram_tensor("adt", [F, E, P], BF16, kind="ExternalInput")
    bd2_d = nc.dram_tensor("bd2", [E * R, D], BF16, kind="ExternalInput")
    oneh_d = nc.dram_tensor("oneh", [E, E, P], BF16, kind="ExternalInput")
    idt_d = nc.dram_tensor("idt", [P, P], F32, kind="ExternalInput")
    out_d = nc.dram_tensor("outT", [D, TC], F32, kind="ExternalOutput")

    with TileContext(nc) as tc:
        with tc.tile_pool(name="big", bufs=1) as big, \
             tc.tile_pool(name="wstream", bufs=3) as wstream, \
             tc.tile_pool(name="xstream", bufs=1) as xstream, \
             tc.tile_pool(name="adtp", bufs=2) as adtp, \
             tc.tile_pool(name="ebuf", bufs=2) as ebuf, \
             tc.tile_pool(name="whbuf", bufs=4) as whbuf, \
             tc.tile_pool(name="gpsbuf", bufs=5) as gpsbuf, \
             tc.tile_pool(name="obuf", bufs=2) as obuf, \
             tc.tile_pool(name="ppg", bufs=3, space="PSUM") as ppg, \
             tc.tile_pool(name="ppu", bufs=2, space="PSUM") as ppu, \
             tc.tile_pool(name="ppt", bufs=1, space="PSUM") as ppt, \
             tc.tile_pool(name="ppk", bufs=1, space="PSUM") as ppk:

            xtr = big.tile([P, DKT, TC], BF16, name="xtr")
            xf8 = big.tile([P, DKT, TC], F8, name="xf8")
            hbar = big.tile([P, FT, TC], BF16, name="hbar")
            wb = big.tile([P, E, TC], BF16, name="wb")
            spm = big.tile([P, E, TC], BF16, name="spm")
            pmt2 = [big.tile([P, 2, E, P], BF16, name=f"pmt{i}") for i in range(2)]
            psb = big.tile([P, TC], BF16, name="psb")
            bd2s = big.tile([P, D], BF16, name="bd2s")
            w8 = big.tile([E, 4, P], BF16, name="w8")
            rw = big.tile([P, DKT, E], F32, name="rw")
            oneh = big.tile([E, E, P], BF16, name="oneh")
            idt = big.tile([P, P], F32, name="idt")
            scr = big.tile([P, 16], F32, name="scr")
            wtl = big.tile([P, 4, E], F32, name="wtl")

            nc.sync.dma_start(rw, rwt_d.rearrange("(kt p) e -> p kt e", p=P))
            nc.sync.dma_start(idt, idt_d[:, :])
            for q in range(2, 4):
                nc.gpsimd.memset(spm[32 * q:32 * (q + 1), :, :], 0.0)
            for i in range(2):
                for q in range(1, 4):
                    nc.gpsimd.memset(pmt2[i][32 * q:32 * (q + 1), 0, :, :], 0.0)
                nc.gpsimd.memset(pmt2[i][0:32, 1, :, :], 0.0)
                for q in range(2, 4):
                    nc.gpsimd.memset(pmt2[i][32 * q:32 * (q + 1), 1, :, :], 0.0)

            pbank = [ppk.tile([P, TC], F32, name=f"pbank{i}") for i in range(2)]

            pslT = ppt.tile([P, TC], F32, name="trans")
            for tt in range(4):
                xt_tt = xstream.tile([P, DKT, P], F32, name="xchunk")
                nc.sync.dma_start(
                    xt_tt, xt_d[:, bass.ts(tt, P)].rearrange("(kt p) t -> p kt t", p=P))
                nc.vector.tensor_copy(xtr[:, :, bass.ts(tt, P)], xt_tt)
                nc.vector.tensor_copy(xf8[:, :, bass.ts(tt, P)], xt_tt)
                for kt in range(DKT):
                    nc.tensor.matmul(pslT[0:E, bass.ts(tt, P)], rw[:, kt, :],
                                     xt_tt[:, kt, :], start=(kt == 0),
                                     stop=(kt == DKT - 1))
            for gi, src in enumerate((agp_d, aup_d)):
                ap_t = xstream.tile([P, DKT, P], BF16, name="apchunk")
                nc.sync.dma_start(ap_t, src.rearrange("(kt p) m -> p kt m", p=P))
                sps = ppu.tile([P, TC], F32, name="banku")
                for kt in range(DKT):
                    nc.tensor.matmul(sps, ap_t[:, kt, :], xtr[:, kt, :],
                                     start=(kt == 0), stop=(kt == DKT - 1))
                s_stage = ebuf.tile([P, TC], BF16, name="t1")
                nc.vector.tensor_copy(s_stage, sps)
                base = 32 * gi
                nc.gpsimd.memset(spm[base:base + R, 0, :], 0.0)
                for e in range(E):
                    if e >= 1:
                        nc.sync.dma_start(spm[base:base + R, e, :],
                                          s_stage[(e - 1) * R:e * R, :])
                    nc.sync.dma_start(spm[base + R:base + 2 * R, e, :],
                                      s_stage[e * R:(e + 1) * R, :])

            lsT = big.tile([E, TC], F32, name="lsT")
            nc.vector.tensor_copy(lsT, pslT[0:E, :])
            for tt in range(4):
                psl = ppt.tile([P, TC], F32, name="trans")
                nc.tensor.transpose(psl[:, 0:E], lsT[:, bass.ts(tt, P)],
                                    idt[0:E, 0:E])
                nmx = scr[:, 1:2]
                mx = scr[:, 0:1]
                m2 = scr[:, 2:3]
                rcp = scr[:, 3:4]
                z = scr[:, 4:12]
                lcp = wtl[:, 0, :]
                nc.vector.tensor_reduce(nmx, psl[:, 0:E], axis=mybir.AxisListType.X,
                                        op=ALU.max, negate=True)
                nc.vector.tensor_scalar_mul(mx, nmx, -1.0)
                nc.scalar.activation(z, psl[:, 0:E], AF.Exp, bias=nmx)
                lm1 = wtl[:, 1, :]
                nc.vector.tensor_scalar(lm1, psl[:, 0:E], mx, -1e30,
                                        op0=ALU.is_ge, op1=ALU.mult)
                nc.vector.tensor_tensor(lcp, psl[:, 0:E], lm1, op=ALU.add)
                nc.vector.tensor_reduce(m2, lcp, axis=mybir.AxisListType.X, op=ALU.max)
                wsel = wtl[:, 1, :]
                nc.vector.scalar_tensor_tensor(wsel, psl[:, 0:E], m2, z,
                                               op0=ALU.is_ge, op1=ALU.mult)
                nc.vector.tensor_reduce(rcp, wsel, axis=mybir.AxisListType.X, op=ALU.add)
                nc.vector.reciprocal(rcp, rcp)
                wcur = wtl[:, 2 + (tt % 2), :]
                nc.vector.tensor_scalar_mul(wcur, wsel, rcp)
                psw = ppt.tile([P, TC], F32, name="trans")
                nc.tensor.transpose(psw[0:E, 0:P], wcur, idt)
                nc.vector.tensor_copy(w8[:, tt, :], psw[0:E, 0:P])
            nc.sync.dma_start(oneh, oneh_d[:, :, :])
            w8flat = w8.rearrange("p a b -> p (a b)")
            for e in range(E):
                pswb = ppt.tile([P, TC], F32, name="trans")
                nc.tensor.matmul(pswb, oneh[:, e, :], w8flat, start=True, stop=True)
                nc.vector.tensor_copy(wb[:, e, :], pswb)

            ldq = []
            adt_last = [None]

            def make_feeder(f):
                wg_t = wstream.tile([P, DKT, P], BF16, name="wchunk")
                nc.sync.dma_start(wg_t, wgt_d[:, bass.ts(f, P)].rearrange(
                    "(kt p) m -> p kt m", p=P))
                wu_t = wstream.tile([P, DKT, P], F8, name="wchunk8")
                nc.sync.dma_start(wu_t, wux_d[f])
                pm_t = pmt2[f % 2]
                nc.sync.dma_start(pm_t[0:32, 0, :, :], pmw_d[0:32, :, bass.ts(f, P)])
                nc.sync.dma_start(pm_t[32:64, 1, :, :], pmw_d[32:64, :, bass.ts(f, P)])
                if f % 2 == 0:
                    adt_t = adtp.tile([P, 2, E, P], BF16, name="adt")
                    nc.sync.dma_start(
                        adt_t, adt_d[f * P:(f + 2) * P, :, :].rearrange(
                            "(fo p) e r -> p fo e r", p=P))
                    adt_last[0] = adt_t
                bank_g = ppg.tile([P, TC], F32, name="bankg")
                bank_u = ppu.tile([P, TC], F32, name="banku")
                steps = []
                for kt in range(DKT):
                    steps.append((bank_g, wg_t[:, kt, :], xtr[:, kt, :],
                                  kt == 0, None))
                steps.append((bank_g, pm_t[:, 0, 0, :], spm[:, 0, :], False, None))
                for i in range(DKT // 2):
                    steps.append((bank_u, wu_t[:, 2 * i:2 * i + 2, :],
                                  xf8[:, 2 * i:2 * i + 2, :], i == 0, DRMODE))
                steps.append((bank_u, pm_t[:, 1, 0, :], spm[:, 0, :], False, None))
                return {"bank_g": bank_g, "bank_u": bank_u, "pm_t": pm_t,
                        "adt": adt_last[0], "steps": steps, "pos": 0}

            def pump(fd, n):
                if fd is None:
                    return
                while n > 0 and fd["pos"] < len(fd["steps"]):
                    out, lhs, rhs, st, pm = fd["steps"][fd["pos"]]
                    fd["pos"] += 1
                    if pm is None:
                        nc.tensor.matmul(out, lhs, rhs, start=st, stop=False)
                    else:
                        nc.tensor.matmul(out, lhs, rhs, start=st, stop=False,
                                         perf_mode=pm)
                    n -= 1

            feeders = {0: make_feeder(0)}
            pump(feeders[0], 999)

            for f in range(FT):
                fd = feeders.pop(f)
                pump(fd, 999)
                fnext = None
                if f + 1 < FT:
                    if f + 1 in feeders:
                        fnext = feeders[f + 1]
                    else:
                        fnext = feeders.setdefault(f + 1, make_feeder(f + 1))
                bank_g, bank_u, pm_t, adt_t = (fd["bank_g"], fd["bank_u"],
                                               fd["pm_t"], fd["adt"])

                wvp = []
                t1p = None
                for e in range(E):
                    s_act = ebuf.tile([P, TC], BF16, name="sact")
                    nc.scalar.activation(s_act, bank_g, AF.Silu)
                    if e + 1 < E:
                        nc.tensor.matmul(bank_g, pm_t[:, 0, e + 1, :],
                                         spm[:, e + 1, :],
                                         start=False, stop=(e + 1 == E - 1))
                    if e % 2 == 0:
                        t1p = whbuf.tile([P, 2, TC], BF16, name="wh")
                    t1 = t1p[:, e % 2, :]
                    nc.vector.scalar_tensor_tensor(t1, bank_u, UPINV, s_act,
                                                   op0=ALU.mult, op1=ALU.mult)
                    if e + 1 < E:
                        nc.tensor.matmul(bank_u, pm_t[:, 1, e + 1, :],
                                         spm[:, e + 1, :],
                                         start=False, stop=(e + 1 == E - 1))
                    ldq.append((pbank[e // 4], adt_t[:, f % 2, e, :], t1,
                                (f == 0 and e % 4 == 0),
                                (f == FT - 1 and e % 4 == 3)))
                    if len(ldq) > 2:
                        args = ldq.pop(0)
                        nc.tensor.matmul(*args[:3], start=args[3], stop=args[4])
                    pump(fnext, 3)
                    if e % 2 == 1:
                        wv = gpsbuf.tile([P, 2, TC], BF16, name="whw")
                        nc.vector.tensor_tensor(wv, t1p, wb[:, e - 1:e + 1, :],
                                                op=ALU.mult)
                        wvp.append(wv)
                a1 = gpsbuf.tile([P, 2, TC], BF16, name="whw")
                nc.gpsimd.tensor_tensor(a1, wvp[0], wvp[1], op=ALU.add)
                a2 = gpsbuf.tile([P, 2, TC], BF16, name="whw")
                nc.gpsimd.tensor_tensor(a2, wvp[2], wvp[3], op=ALU.add)
                nc.gpsimd.tensor_tensor(a1, a1, a2, op=ALU.add)
                nc.gpsimd.tensor_tensor(hbar[:, f, :], a1[:, 0, :], a1[:, 1, :],
                                        op=ALU.add)

            for args in ldq:
                nc.tensor.matmul(*args[:3], start=args[3], stop=args[4])
            ldq.clear()

            for b in range(2):
                p_stage = ebuf.tile([P, TC], BF16, name="t1")
                for eo in range(4):
                    e = b * 4 + eo
                    nc.vector.scalar_tensor_tensor(
                        p_stage[32 * eo:32 * eo + R, :],
                        pbank[b][32 * eo:32 * eo + R, :], 1.0,
                        wb[0:R, e, :], op0=ALU.bypass, op1=ALU.mult)
                    nc.sync.dma_start(psb[e * R:(e + 1) * R, :],
                                      p_stage[32 * eo:32 * eo + R, :])

            nc.sync.dma_start(bd2s, bd2_d[:, :])
            for d in range(DT_TILES):
                psd = ppg.tile([P, TC], F32, name="bankg")
                for fc in range(4):
                    wd_t = wstream.tile([P, DKT, P], BF16, name="wdchunk")
                    nc.sync.dma_start(
                        wd_t, wdt_d[fc * 2048:(fc + 1) * 2048, bass.ts(d, P)].rearrange(
                            "(kt p) m -> p kt m", p=P))
                    for kt in range(DKT):
                        nc.tensor.matmul(psd, wd_t[:, kt, :], hbar[:, fc * DKT + kt, :],
                                         start=(fc == 0 and kt == 0), stop=False)
                nc.tensor.matmul(psd, bd2s[:, bass.ts(d, P)], psb,
                                 start=False, stop=True)
                o_t = obuf.tile([P, TC], F32, name="osb")
                nc.scalar.activation(o_t, psd, AF.Copy)
                nc.sync.dma_start(out_d[bass.ts(d, P), :], o_t)

    nc.finalize()
    _NC_CACHE['nc'] = nc
    return nc


def _host_prep(hidden_states, router_w, Wg, Wu, Wd, Ag, Bg, Au, Bu, Ad, Bd):
    f32 = np.float32
    X = np.ascontiguousarray(hidden_states.reshape(T_FULL, D), dtype=f32)
    xT = np.ascontiguousarray(X.T)
    wus = (np.ascontiguousarray(Wu.T, dtype=f32) * UPSCALE).astype(F8NP)
    wux = np.ascontiguousarray(
        wus.reshape(DKT, P, FT, P).transpose(2, 1, 0, 3))
    shared = {
        "wgt": np.ascontiguousarray(Wg.T).astype(BF16NP),
        "wux": wux,
        "wdt": np.ascontiguousarray(Wd.T).astype(BF16NP),
        "rwt": np.ascontiguousarray(router_w.T, dtype=f32),
        "agp": np.ascontiguousarray(Ag.transpose(2, 0, 1).reshape(D, E * R)).astype(BF16NP),
        "aup": np.ascontiguousarray(Au.transpose(2, 0, 1).reshape(D, E * R)).astype(BF16NP),
    }
    pmw = np.zeros((64, E, F), dtype=f32)
    BgT = np.transpose(Bg, (0, 2, 1))
    BuT = np.transpose(Bu, (0, 2, 1))
    for e in range(E):
        if e >= 1:
            pmw[0:R, e] = -2.0 * BgT[e - 1]
            pmw[32:48, e] = -2.0 * UPSCALE * BuT[e - 1]
        pmw[R:32, e] = 2.0 * BgT[e]
        pmw[48:64, e] = 2.0 * UPSCALE * BuT[e]
    shared["pmw"] = pmw.astype(BF16NP)
    adt = np.zeros((F, E, P), dtype=f32)
    AdT = Ad.transpose(2, 0, 1)
    for e in range(E):
        adt[:, e, 32 * (e % 4):32 * (e % 4) + R] = AdT[:, e, :]
    shared["adt"] = adt.astype(BF16NP)
    shared["bd2"] = np.ascontiguousarray(
        (2.0 * Bd.transpose(0, 2, 1)).reshape(E * R, D)).astype(BF16NP)
    oneh = np.zeros((E, E, P), dtype=f32)
    for e in range(E):
        oneh[e, e, :] = 1.0
    shared["oneh"] = oneh.astype(BF16NP)
    shared["idt"] = np.eye(P, dtype=f32)
    in_maps = []
    for c in range(NCORES):
        m = dict(shared)
        m["xt"] = np.ascontiguousarray(xT[:, c * TC:(c + 1) * TC])
        in_maps.append(m)
    return in_maps


def kernel(hidden_states, router_w, Wg, Wu, Wd, Ag, Bg, Au, Bu, Ad, Bd):
    hidden_states = np.asarray(hidden_states)
    nc = build_nc()
    in_maps = _host_prep(np.asarray(hidden_states, dtype=np.float32),
                         np.asarray(router_w), np.asarray(Wg), np.asarray(Wu),
                         np.asarray(Wd), np.asarray(Ag), np.asarray(Bg),
                         np.asarray(Au), np.asarray(Bu), np.asarray(Ad),
                         np.asarray(Bd))
    trace = bool(os.environ.get("TRNK_TRACE"))
    res = bass_utils.run_bass_kernel_spmd(
        nc, in_maps, core_ids=list(range(NCORES)), trace=trace)
    LAST_RESULT['exec_time_ns'] = res.exec_time_ns
    LAST_RESULT['res'] = res
    out = np.empty((T_FULL, D), dtype=np.float32)
    for c in range(NCORES):
        out[c * TC:(c + 1) * TC, :] = res.results[c]["outT"].T
    return out.reshape(hidden_states.shape[0], hidden_states.shape[1], D)

